# revision 34
# baseline (speedup 1.0000x reference)
"""Multi-head graph-attention (GAT) kernel for Trainium2, 8 NeuronCores.

Reference computation (per head):
    h_prime = h @ w[head]                       # [8192, 64]
    s = h_prime @ a_src[head],  d = h_prime @ a_dst[head]
    attn = softmax_j(leaky_relu(s_i + d_j, 0.2))
    out  = attn @ h_prime + bias                # -> [8192, 4*64]

Key identity: with exp monotone,
    exp(lrelu(s_i + d_j)) = e^{s_i} e^{d_j}           if s_i + d_j >= 0
                          = e^{0.2 s_i} e^{0.2 d_j}   otherwise
The mask sets {j : d_j >= -s_i} are NESTED across i, so the whole O(n^2)
attention contraction is a 1-D step function of t = -s_i:
    g(t) = sum_{j: d_j >= t} [v_j h'_j | v_j],   v = e^d   (and gq with q)
Evaluate g on a fixed grid of B=128 thresholds t_b (one fp16 matmul with
only B moving columns per 128-row j-tile), then each query i picks its
column b_i = clamp((-s_i - LO)/G) via a one-hot matmul that lands the
result directly in query-partition layout.  Grid rounding only
misclassifies js with |s_i + d_j| <= G, where exp(lrelu(x)) is
continuous, so the error is O(G^2) ~ 3e-4.  Everything else is exact:
    num_i / e^{s_i} = A(b_i) - r_i (C(b_i) - Sq),   r_i = e^{-0.8 s_i}
    den_i / e^{s_i} = a(b_i) - r_i (c(b_i) - sq)
with A|a = g_v columns, C|c = g_q columns, Sq|sq the full q-sums
(= g_q at the leftmost grid point).

Sharding: 8 cores = 4 heads x 2 query-halves. Each core gets the full h
(rows rotated so its query half is rows 0..4095 - j-side sums are order
invariant), computes the full j-side tables, and evaluates its 4096
queries. No collectives.
"""

import numpy as np

import concourse.bass as bass
import concourse.tile as tile
from concourse import bacc, mybir
from concourse.bass_utils import run_bass_kernel_spmd
from concourse.masks import make_identity

F32 = mybir.dt.float32
F32R = mybir.dt.float32r
BF16 = mybir.dt.bfloat16
FP16 = mybir.dt.float16
I32 = mybir.dt.int32
AF = mybir.ActivationFunctionType
OP = mybir.AluOpType

BS = 8192          # nodes
F = 64             # f_in == f_out
NH = 4             # heads
HALF = BS // 2     # queries per core
NT_J = BS // 128   # 64 j tiles
NT_I = HALF // 128 # 32 query tiles
ALPHA = 0.2

BGRID = 128        # grid points (one PE tile)
LO = -8.0          # grid range [LO, -LO)
G = (-2.0 * LO) / BGRID   # 1/8
INVG = 1.0 / G

AUGW = 2 * (F + 1)  # 130: [Hv|v | Hq|q] stationary width


def _build_kernel_module():
    nc = bacc.Bacc("TRN2", target_bir_lowering=False, debug=False)

    hfull_d = nc.dram_tensor("hfull", [BS, F], F32, kind="ExternalInput")
    w_d = nc.dram_tensor("w", [F, F], F32, kind="ExternalInput")
    aa_d = nc.dram_tensor("aa", [F, 2], F32, kind="ExternalInput")
    bias_d = nc.dram_tensor("bias", [1, F], F32, kind="ExternalInput")
    out_d = nc.dram_tensor("out", [HALF, F], F32, kind="ExternalOutput")

    with tile.TileContext(nc) as tc:
        with (
            tc.tile_pool(name="const", bufs=1) as cpool,
            tc.tile_pool(name="work", bufs=3) as wpool,
            tc.tile_pool(name="dscr", bufs=1, space="DRAM") as dpool,
            tc.tile_pool(name="psum", bufs=2, space="PSUM") as ppool,
        ):
            # ---------------- constants ----------------
            identity = cpool.tile([128, 128], F32)
            make_identity(nc, identity[:])
            ones = cpool.tile([128, 128], F32)
            nc.gpsimd.memset(ones[:], 1.0)
            iota_col = cpool.tile([128, 1], I32)
            nc.gpsimd.iota(iota_col[:], [[1, 1]], base=0, channel_multiplier=1)
            iota_f = cpool.tile([128, 1], F32)
            nc.vector.tensor_copy(iota_f[:], iota_col[:])

            # ---------------- h load, h^T (paired), h' (+d +s) -------------
            # hT2 chunk p holds transposed tiles 2p (partitions 0:64) and
            # 2p+1 (partitions 64:128).
            hT2 = cpool.tile([128, (NT_J // 2) * 128], BF16)
            # h' in bf16, padded with a ones column (aug col 64)
            hprB = cpool.tile([128, NT_J * (F + 1)], BF16)
            hprB3 = hprB[:].rearrange("p (t c) -> p t c", c=F + 1)
            nc.gpsimd.memset(hprB3[:, :, F], 1.0)
            ds_col = cpool.tile([128, NT_J * 2], F32)  # [d | s] per tile
            ds3 = ds_col[:].rearrange("p (t c) -> p t c", c=2)
            hf_view = hfull_d.ap().rearrange("(a p) f -> p a f", p=128)
            for blk in range(NT_J // 8):
                ldb = wpool.tile([128, 8 * F], F32, tag="hloadb", bufs=3)
                nc.sync.dma_start(
                    ldb[:], hf_view[:, blk * 8 : (blk + 1) * 8, :]
                )
                for k in range(4):
                    pr = blk * 4 + k  # pair index
                    tr = ppool.tile([128, 128], F32, tag="mix")
                    nc.tensor.transpose(
                        tr[:],
                        ldb[:, k * 128 : (k + 1) * 128],
                        identity[:],
                    )
                    if k % 2 == 0:
                        nc.scalar.copy(hT2[:, pr * 128 : (pr + 1) * 128], tr[:])
                    else:
                        nc.vector.tensor_copy(
                            hT2[:, pr * 128 : (pr + 1) * 128], tr[:]
                        )
            # ---------------- tiny weight prep ----------------
            w_sb = cpool.tile([F, F], F32)
            nc.sync.dma_start(w_sb[:], w_d.ap())
            aa_sb = cpool.tile([F, 2], F32)
            nc.sync.dma_start(aa_sb[:], aa_d.ap())
            bias_sb = cpool.tile([1, F], F32)
            nc.sync.dma_start(bias_sb[:], bias_d.ap())

            wT_ps = ppool.tile([F, F], F32, tag="mix")
            nc.tensor.transpose(wT_ps[:], w_sb[:], identity[0:F, 0:F])
            wT_sb = cpool.tile([F, F], F32)
            nc.scalar.copy(wT_sb[:], wT_ps[:])

            # ws = w @ [a_src | a_dst]  -> [64, 2]
            ws_ps = ppool.tile([F, 2], F32, tag="mix")
            nc.tensor.matmul(ws_ps[:], wT_sb[:], aa_sb[:])
            ws_sb = cpool.tile([F, 2], F32)
            nc.scalar.copy(ws_sb[:], ws_ps[:])

            # w_aug = [w | w@a_dst | w@a_src]: h @ w_aug -> [h' | d | s]
            # replicated on partitions 64:128 for odd-tile stationaries
            w_top = cpool.tile([F, F + 2], F32)
            nc.scalar.copy(w_top[:, 0:F], w_sb[:])
            nc.scalar.copy(w_top[:, F : F + 1], ws_sb[:, 1:2])
            nc.scalar.copy(w_top[:, F + 1 : F + 2], ws_sb[:, 0:1])
            dbl_id = cpool.tile([F, 128], F32)
            nc.scalar.copy(dbl_id[:, 0:F], identity[0:F, 0:F])
            nc.vector.tensor_copy(dbl_id[:, F:128], identity[0:F, 0:F])
            wa_ps = ppool.tile([128, F + 2], F32, tag="mix")
            nc.tensor.matmul(wa_ps[:], dbl_id[:], w_top[:])
            w_aug = cpool.tile([128, F + 2], BF16)
            nc.scalar.copy(w_aug[:], wa_ps[:])

            # bias broadcast to all partitions
            biasb_ps = ppool.tile([128, F], F32, tag="mix")
            nc.tensor.matmul(biasb_ps[:], ones[0:1, 0:128], bias_sb[:])
            bias_rep = cpool.tile([128, F], F32)
            nc.scalar.copy(bias_rep[:], biasb_ps[:])

            # grid thresholds replicated across partitions, bf16-exact
            tg_f = cpool.tile([1, BGRID], F32)
            nc.gpsimd.iota(
                tg_f[:], [[1, BGRID]], base=0, channel_multiplier=0,
                allow_small_or_imprecise_dtypes=True,
            )
            tgrid = cpool.tile([1, BGRID], F32)
            nc.vector.tensor_scalar(
                tgrid[:], tg_f[:], G, LO, op0=OP.mult, op1=OP.add
            )
            trow_ps = ppool.tile([128, BGRID], F32, tag="mix")
            nc.tensor.matmul(trow_ps[:], ones[0:1, 0:128], tgrid[:])
            trow_b = cpool.tile([128, BGRID], BF16)
            nc.scalar.copy(trow_b[:], trow_ps[:])

            for jt in range(NT_J):
                pr, par = jt // 2, (jt % 2) * F
                hp_ps = ppool.tile([128, F + 2], F32, tag="mix")
                nc.tensor.matmul(
                    hp_ps[:],
                    hT2[par : par + F, pr * 128 : (pr + 1) * 128],
                    w_aug[par : par + F, :],
                )
                if jt % 2 == 0:
                    nc.scalar.copy(hprB3[:, jt, 0:F], hp_ps[:, 0:F])
                    nc.vector.tensor_copy(ds3[:, jt, :], hp_ps[:, F : F + 2])
                else:
                    nc.vector.tensor_copy(hprB3[:, jt, 0:F], hp_ps[:, 0:F])
                    nc.scalar.copy(ds3[:, jt, :], hp_ps[:, F : F + 2])

            d_all = ds3[:, :, 0]            # [128, NT_J] strided
            s_half = ds3[:, 0:NT_I, 1]      # queries = rows 0..4095

            v_col = cpool.tile([128, NT_J], F32)
            q_col = cpool.tile([128, NT_J], F32)

            # ---------------- query-side prep ----------------
            negr_col = cpool.tile([128, NT_I], F32)
            nc.scalar.activation(negr_col[:], s_half, AF.Exp, scale=-(1.0 - ALPHA))
            nc.vector.tensor_scalar_mul(negr_col[:], negr_col[:], -1.0)

            b_f = wpool.tile([128, NT_I], F32, tag="bf", bufs=1)
            nc.vector.tensor_scalar(
                b_f[:], s_half, -INVG, -LO * INVG, op0=OP.mult, op1=OP.add
            )
            b_rnd = wpool.tile([128, NT_I], F32, tag="brnd", bufs=1)
            nc.vector.tensor_scalar(
                b_rnd[:], b_f[:], 8388608.0, 8388608.0,
                op0=OP.add, op1=OP.subtract,
            )
            b_cl = wpool.tile([128, NT_I], BF16, tag="bcl", bufs=1)
            nc.vector.tensor_scalar(
                b_cl[:], b_rnd[:], 0.0, float(BGRID - 1), op0=OP.max, op1=OP.min
            )
            negr16 = wpool.tile([128, NT_I], FP16, tag="negr16", bufs=1)
            nc.vector.tensor_copy(negr16[:], negr_col[:])
            r_dram = dpool.tile([HALF], FP16, name="rscr")
            nc.sync.dma_start(
                r_dram[:].rearrange("(c q) -> q c", q=128), negr16[:]
            )
            negr_row = cpool.tile([1, HALF], FP16)
            nc.sync.dma_start(
                negr_row[:], r_dram[:].rearrange("(a n) -> a n", a=1)
            )
            ones_fp = cpool.tile([1, 128], FP16)
            nc.vector.tensor_copy(ones_fp[:], ones[0:1, 0:128])

            # roundtrip through DRAM to get b as a single row [1, 4096]
            b_dram = dpool.tile([HALF], BF16, name="bscr")
            nc.sync.dma_start(
                b_dram[:].rearrange("(c q) -> q c", q=128), b_cl[:]
            )
            b_row = cpool.tile([1, HALF], BF16)
            ones_bf = cpool.tile([1, 128], BF16)
            nc.vector.tensor_copy(ones_bf[:], ones[0:1, 0:128])
            nc.sync.dma_start(b_row[:], b_dram[:].rearrange("(a n) -> a n", a=1))

            # ------- fused: exps + ST build + mask + grid matmul -------
            bankV = ppool.tile([F + 1, BGRID], F32, tag="bankV", bufs=1)
            bankQ = ppool.tile([F + 1, BGRID], F32, tag="bankQ", bufs=1)
            GRP = 16
            for jt in range(NT_J):
                if jt % GRP == 0:
                    gs = slice(jt, jt + GRP)
                    nc.scalar.activation(v_col[:, gs], d_all[:, gs], AF.Exp)
                    nc.scalar.activation(
                        q_col[:, gs], d_all[:, gs], AF.Exp, scale=ALPHA
                    )
                st_t = wpool.tile([128, AUGW], FP16, tag="stt", bufs=4)
                if jt % 4 == 3:
                    nc.scalar.activation(
                        st_t[:, 0 : F + 1], hprB3[:, jt, :], AF.Identity,
                        scale=v_col[:, jt : jt + 1],
                    )
                    nc.scalar.activation(
                        st_t[:, F + 1 : AUGW], hprB3[:, jt, :],
                        AF.Identity, scale=q_col[:, jt : jt + 1],
                    )
                else:
                    nc.vector.tensor_scalar_mul(
                        st_t[:, 0 : F + 1], hprB3[:, jt, :],
                        v_col[:, jt : jt + 1],
                    )
                    nc.vector.tensor_scalar_mul(
                        st_t[:, F + 1 : AUGW], hprB3[:, jt, :],
                        q_col[:, jt : jt + 1],
                    )
                mw = wpool.tile([128, BGRID], FP16, tag="mask", bufs=4)
                nc.vector.tensor_scalar(
                    mw[:], trow_b[:], ds3[:, jt, 0:1], None, op0=OP.is_le
                )
                st, sp = (jt == 0), (jt == NT_J - 1)
                nc.tensor.matmul(
                    bankV[:], st_t[:, 0 : F + 1], mw[:], start=st, stop=sp
                )
                nc.tensor.matmul(
                    bankQ[:], st_t[:, F + 1 : AUGW], mw[:], start=st, stop=sp
                )

            # one-hot of query buckets: oh[b, i] = (b_i == b), fp16
            # and ohr = oh * (-r_i) (pre-scaled for the D-side matmul)
            oh = cpool.tile([128, HALF], FP16)
            ohr = cpool.tile([128, HALF], FP16)
            for ch in range(8):
                br_ps = ppool.tile([128, 512], F32, tag="mix")
                nc.tensor.matmul(
                    br_ps[:],
                    ones_bf[:],
                    b_row[:, ch * 512 : (ch + 1) * 512],
                )
                nc.vector.tensor_scalar(
                    oh[:, ch * 512 : (ch + 1) * 512], br_ps[:],
                    iota_f[:, 0:1], None, op0=OP.is_equal,
                )
                nr_ps = ppool.tile([128, 512], F32, tag="mix")
                nc.tensor.matmul(
                    nr_ps[:],
                    ones_fp[:],
                    negr_row[:, ch * 512 : (ch + 1) * 512],
                )
                nc.vector.tensor_tensor(
                    ohr[:, ch * 512 : (ch + 1) * 512],
                    oh[:, ch * 512 : (ch + 1) * 512], nr_ps[:],
                    op=OP.mult,
                )

            # tables: A = g_v, D = g_q - Sq  (Sq = g_q[:, 0]); transpose to
            # [grid-part, comp] fp16 for the one-hot matmuls
            sq_col = cpool.tile([F + 1, 1], F32)
            nc.vector.tensor_copy(sq_col[:], bankQ[:, 0:1])
            A_sb = cpool.tile([F + 1, BGRID], F32)
            nc.scalar.copy(A_sb[:], bankV[:])
            D_sb = cpool.tile([F + 1, BGRID], F32)
            nc.vector.tensor_scalar(
                D_sb[:], bankQ[:], sq_col[:, 0:1], None, op0=OP.subtract
            )
            gvT_ps = ppool.tile([BGRID, F + 1], F32, tag="mix")
            nc.tensor.transpose(
                gvT_ps[:], A_sb[:],
                identity[0 : F + 1, 0 : F + 1],
            )
            gvd = cpool.tile([BGRID, AUGW], FP16)
            nc.scalar.copy(gvd[:, 0 : F + 1], gvT_ps[:])
            gdT_ps = ppool.tile([BGRID, F + 1], F32, tag="mix")
            nc.tensor.transpose(
                gdT_ps[:], D_sb[:],
                identity[0 : F + 1, 0 : F + 1],
            )
            nc.vector.tensor_copy(gdT_ps_sb_dummy := gvd[:, F + 1 : AUGW], gdT_ps[:])

            # ---------------- per-tile one-hot gather + epilogue -----------
            o_all = cpool.tile([128, NT_I * F], F32)
            o3 = o_all[:].rearrange("p (t c) -> p t c", c=F)
            out_view = out_d.ap().rearrange("(a p) f -> p a f", p=128)
            for it in range(NT_I):
                if it % 2 == 0:
                    # PE-heavy path: A - r*D accumulated in PSUM
                    ad_ps = ppool.tile([128, F + 1], F32, tag="adps", bufs=2)
                    nc.tensor.matmul(
                        ad_ps[:], oh[:, it * 128 : (it + 1) * 128],
                        gvd[:, 0 : F + 1], start=True, stop=False,
                    )
                    nc.tensor.matmul(
                        ad_ps[:], ohr[:, it * 128 : (it + 1) * 128],
                        gvd[:, F + 1 : AUGW], start=False, stop=True,
                    )
                    rec = wpool.tile([128, 1], F32, tag="rec", bufs=3)
                    nc.vector.reciprocal(rec[:], ad_ps[:, F : F + 1])
                    nc.vector.scalar_tensor_tensor(
                        o3[:, it, :], ad_ps[:, 0:F], rec[:, 0:1],
                        bias_rep[:], op0=OP.mult, op1=OP.add,
                    )
                else:
                    # ACT-heavy path: single matmul, scalar epilogue
                    ad2_ps = ppool.tile([128, AUGW], F32, tag="adps2", bufs=2)
                    nc.tensor.matmul(
                        ad2_ps[:], oh[:, it * 128 : (it + 1) * 128], gvd[:]
                    )
                    a_sb = wpool.tile([128, F + 1], F32, tag="asb", bufs=2)
                    nc.scalar.copy(a_sb[:], ad2_ps[:, 0 : F + 1])
                    num = wpool.tile([128, F], F32, tag="num", bufs=2)
                    nc.vector.scalar_tensor_tensor(
                        num[:], ad2_ps[:, F + 1 : AUGW - 1],
                        negr_col[:, it : it + 1], a_sb[:, 0:F],
                        op0=OP.mult, op1=OP.add,
                    )
                    den = wpool.tile([128, 1], F32, tag="den", bufs=2)
                    nc.scalar.activation(
                        den[:], ad2_ps[:, AUGW - 1 : AUGW], AF.Identity,
                        scale=negr_col[:, it : it + 1],
                        bias=a_sb[:, F : F + 1],
                    )
                    rec = wpool.tile([128, 1], F32, tag="rec2", bufs=2)
                    nc.vector.reciprocal(rec[:], den[:])
                    nc.vector.scalar_tensor_tensor(
                        o3[:, it, :], num[:], rec[:, 0:1], bias_rep[:],
                        op0=OP.mult, op1=OP.add,
                    )
                if it % 4 == 3:
                    grp = it // 4
                    nc.sync.dma_start(
                        out_view[:, grp * 4 : (grp + 1) * 4, :],
                        o_all[:, grp * 4 * F : (grp + 1) * 4 * F],
                    )

    nc.compile()
    return nc


_NC_CACHE = None


def _get_nc():
    global _NC_CACHE
    if _NC_CACHE is None:
        _NC_CACHE = _build_kernel_module()
    return _NC_CACHE


def _make_in_maps(h, w, a_src, a_dst, bias):
    h = np.ascontiguousarray(np.asarray(h, dtype=np.float32))
    w = np.asarray(w, dtype=np.float32)
    a_src = np.asarray(a_src, dtype=np.float32)
    a_dst = np.asarray(a_dst, dtype=np.float32)
    bias = np.asarray(bias, dtype=np.float32).reshape(1, F)
    in_maps = []
    for c in range(8):
        head, half = c // 2, c % 2
        aa = np.ascontiguousarray(
            np.concatenate([a_src[head], a_dst[head]], axis=1)
        )
        # rotate rows so this core's query half is rows 0..HALF-1
        hrot = np.ascontiguousarray(
            np.concatenate([h[half * HALF :], h[: half * HALF]])
        )
        in_maps.append(
            {
                "hfull": hrot,
                "w": np.ascontiguousarray(w[head]),
                "aa": aa,
                "bias": bias,
            }
        )
    return in_maps


def _run(h, w, a_src, a_dst, bias, trace=False, **trace_kwargs):
    nc = _get_nc()
    in_maps = _make_in_maps(h, w, a_src, a_dst, bias)
    res = run_bass_kernel_spmd(
        nc, in_maps, core_ids=list(range(8)), trace=trace, **trace_kwargs
    )
    out = np.zeros((BS, NH * F), dtype=np.float32)
    for c in range(8):
        head, half = c // 2, c % 2
        out[half * HALF : (half + 1) * HALF, head * F : (head + 1) * F] = res.results[
            c
        ]["out"]
    return out, res


def kernel(h, w, a_src, a_dst, bias):
    out, _ = _run(h, w, a_src, a_dst, bias, trace=False)
    return out


# revision 36
# speedup vs baseline: 1.1801x; 1.1801x over previous
"""Multi-head graph-attention (GAT) kernel for Trainium2, 8 NeuronCores.

Reference computation (per head):
    h_prime = h @ w[head]                       # [8192, 64]
    s = h_prime @ a_src[head],  d = h_prime @ a_dst[head]
    attn = softmax_j(leaky_relu(s_i + d_j, 0.2))
    out  = attn @ h_prime + bias                # -> [8192, 4*64]

Key identity: with exp monotone,
    exp(lrelu(s_i + d_j)) = e^{s_i} e^{d_j}           if s_i + d_j >= 0
                          = e^{0.2 s_i} e^{0.2 d_j}   otherwise
The mask sets {j : d_j >= -s_i} are NESTED across i, so the whole O(n^2)
attention contraction is a 1-D step function of t = -s_i:
    g(t) = sum_{j: d_j >= t} [v_j h'_j | v_j],   v = e^d   (and gq with q)
Evaluate g on a fixed grid of B=128 thresholds t_b (one fp16 matmul with
only B moving columns per 128-row j-tile), then each query i picks its
column b_i = clamp((-s_i - LO)/G) via a one-hot matmul that lands the
result directly in query-partition layout.  Grid rounding only
misclassifies js with |s_i + d_j| <= G, where exp(lrelu(x)) is
continuous, so the error is O(G^2) ~ 3e-4.  Everything else is exact:
    num_i / e^{s_i} = A(b_i) - r_i (C(b_i) - Sq),   r_i = e^{-0.8 s_i}
    den_i / e^{s_i} = a(b_i) - r_i (c(b_i) - sq)
with A|a = g_v columns, C|c = g_q columns, Sq|sq the full q-sums
(= g_q at the leftmost grid point).

Sharding: 8 cores = 4 heads x 2 query-halves. Each core gets the full h
(rows rotated so its query half is rows 0..4095 - j-side sums are order
invariant), computes the full j-side tables, and evaluates its 4096
queries. No collectives.
"""

import numpy as np

import concourse.bass as bass
import concourse.tile as tile
from concourse import bacc, mybir
from concourse.bass_utils import run_bass_kernel_spmd
from concourse.masks import make_identity

F32 = mybir.dt.float32
F32R = mybir.dt.float32r
BF16 = mybir.dt.bfloat16
FP16 = mybir.dt.float16
I32 = mybir.dt.int32
AF = mybir.ActivationFunctionType
OP = mybir.AluOpType

BS = 8192          # nodes
F = 64             # f_in == f_out
NH = 4             # heads
HALF = BS // 2     # queries per core
NT_J = BS // 128   # 64 j tiles
NT_I = HALF // 128 # 32 query tiles
ALPHA = 0.2

BGRID = 128        # grid points (one PE tile)
LO = -8.0          # grid range [LO, -LO)
G = (-2.0 * LO) / BGRID   # 1/8
INVG = 1.0 / G

AUGW = 2 * (F + 1)  # 130: [Hv|v | Hq|q] stationary width


def _build_kernel_module():
    nc = bacc.Bacc("TRN2", target_bir_lowering=False, debug=False)

    hfull_d = nc.dram_tensor("hfull", [BS, F], F32, kind="ExternalInput")
    w_d = nc.dram_tensor("w", [F, F], F32, kind="ExternalInput")
    aa_d = nc.dram_tensor("aa", [F, 2], F32, kind="ExternalInput")
    bias_d = nc.dram_tensor("bias", [1, F], F32, kind="ExternalInput")
    out_d = nc.dram_tensor("out", [HALF, F], F32, kind="ExternalOutput")

    with tile.TileContext(nc) as tc:
        with (
            tc.tile_pool(name="const", bufs=1) as cpool,
            tc.tile_pool(name="work", bufs=3) as wpool,
            tc.tile_pool(name="dscr", bufs=1, space="DRAM") as dpool,
            tc.tile_pool(name="psum", bufs=3, space="PSUM") as ppool,
        ):
            # ---------------- constants ----------------
            identity = cpool.tile([128, 128], F32)
            make_identity(nc, identity[:])
            ones = cpool.tile([128, 128], F32)
            nc.gpsimd.memset(ones[:], 1.0)
            iota_col = cpool.tile([128, 1], I32)
            nc.gpsimd.iota(iota_col[:], [[1, 1]], base=0, channel_multiplier=1)
            iota_f = cpool.tile([128, 1], F32)
            nc.vector.tensor_copy(iota_f[:], iota_col[:])

            # ---------------- h load, h^T (paired), h' (+d +s) -------------
            # hT2 chunk p holds transposed tiles 2p (partitions 0:64) and
            # 2p+1 (partitions 64:128).
            hT2 = cpool.tile([128, (NT_J // 2) * 128], BF16)
            # h' in bf16, padded with a ones column (aug col 64)
            hprB = cpool.tile([128, NT_J * (F + 1)], BF16)
            hprB3 = hprB[:].rearrange("p (t c) -> p t c", c=F + 1)
            nc.gpsimd.memset(hprB3[:, :, F], 1.0)
            ds_col = cpool.tile([128, NT_J * 2], F32)  # [d | s] per tile
            ds3 = ds_col[:].rearrange("p (t c) -> p t c", c=2)
            hf_view = hfull_d.ap().rearrange("(a p) f -> p a f", p=128)
            for blk in range(NT_J // 8):
                ldb = wpool.tile([128, 8 * F], F32, tag="hloadb", bufs=3)
                nc.sync.dma_start(
                    ldb[:], hf_view[:, blk * 8 : (blk + 1) * 8, :]
                )
                for k in range(4):
                    pr = blk * 4 + k  # pair index
                    tr = ppool.tile([128, 128], F32, tag="mix")
                    nc.tensor.transpose(
                        tr[:],
                        ldb[:, k * 128 : (k + 1) * 128],
                        identity[:],
                    )
                    if k % 2 == 0:
                        nc.scalar.copy(hT2[:, pr * 128 : (pr + 1) * 128], tr[:])
                    else:
                        nc.vector.tensor_copy(
                            hT2[:, pr * 128 : (pr + 1) * 128], tr[:]
                        )
            # ---------------- tiny weight prep ----------------
            w_sb = cpool.tile([F, F], F32)
            nc.sync.dma_start(w_sb[:], w_d.ap())
            aa_sb = cpool.tile([F, 2], F32)
            nc.sync.dma_start(aa_sb[:], aa_d.ap())
            bias_sb = cpool.tile([1, F], F32)
            nc.sync.dma_start(bias_sb[:], bias_d.ap())

            wT_ps = ppool.tile([F, F], F32, tag="mix")
            nc.tensor.transpose(wT_ps[:], w_sb[:], identity[0:F, 0:F])
            wT_sb = cpool.tile([F, F], F32)
            nc.scalar.copy(wT_sb[:], wT_ps[:])

            # ws = w @ [a_src | a_dst]  -> [64, 2]
            ws_ps = ppool.tile([F, 2], F32, tag="mix")
            nc.tensor.matmul(ws_ps[:], wT_sb[:], aa_sb[:])
            ws_sb = cpool.tile([F, 2], F32)
            nc.scalar.copy(ws_sb[:], ws_ps[:])

            # w_aug = [w | w@a_dst | w@a_src]: h @ w_aug -> [h' | d | s]
            # replicated on partitions 64:128 for odd-tile stationaries
            w_top = cpool.tile([F, F + 2], F32)
            nc.scalar.copy(w_top[:, 0:F], w_sb[:])
            nc.scalar.copy(w_top[:, F : F + 1], ws_sb[:, 1:2])
            nc.scalar.copy(w_top[:, F + 1 : F + 2], ws_sb[:, 0:1])
            dbl_id = cpool.tile([F, 128], F32)
            nc.scalar.copy(dbl_id[:, 0:F], identity[0:F, 0:F])
            nc.vector.tensor_copy(dbl_id[:, F:128], identity[0:F, 0:F])
            wa_ps = ppool.tile([128, F + 2], F32, tag="mix")
            nc.tensor.matmul(wa_ps[:], dbl_id[:], w_top[:])
            w_aug = cpool.tile([128, F + 2], BF16)
            nc.scalar.copy(w_aug[:], wa_ps[:])

            # bias broadcast to all partitions
            biasb_ps = ppool.tile([128, F], F32, tag="mix")
            nc.tensor.matmul(biasb_ps[:], ones[0:1, 0:128], bias_sb[:])
            bias_rep = cpool.tile([128, F], F32)
            nc.scalar.copy(bias_rep[:], biasb_ps[:])

            # grid thresholds replicated across partitions, bf16-exact
            tg_f = cpool.tile([1, BGRID], F32)
            nc.gpsimd.iota(
                tg_f[:], [[1, BGRID]], base=0, channel_multiplier=0,
                allow_small_or_imprecise_dtypes=True,
            )
            tgrid = cpool.tile([1, BGRID], F32)
            nc.vector.tensor_scalar(
                tgrid[:], tg_f[:], G, LO, op0=OP.mult, op1=OP.add
            )
            trow_ps = ppool.tile([128, BGRID], F32, tag="mix")
            nc.tensor.matmul(trow_ps[:], ones[0:1, 0:128], tgrid[:])
            trow_b = cpool.tile([128, BGRID], BF16)
            nc.scalar.copy(trow_b[:], trow_ps[:])

            for jt in range(NT_J):
                pr, par = jt // 2, (jt % 2) * F
                hp_ps = ppool.tile([128, F + 2], F32, tag="mix")
                nc.tensor.matmul(
                    hp_ps[:],
                    hT2[par : par + F, pr * 128 : (pr + 1) * 128],
                    w_aug[par : par + F, :],
                )
                if jt % 2 == 0:
                    nc.scalar.copy(hprB3[:, jt, 0:F], hp_ps[:, 0:F])
                    nc.vector.tensor_copy(ds3[:, jt, :], hp_ps[:, F : F + 2])
                else:
                    nc.vector.tensor_copy(hprB3[:, jt, 0:F], hp_ps[:, 0:F])
                    nc.scalar.copy(ds3[:, jt, :], hp_ps[:, F : F + 2])

            d_all = ds3[:, :, 0]            # [128, NT_J] strided
            s_half = ds3[:, 0:NT_I, 1]      # queries = rows 0..4095

            v_col = cpool.tile([128, NT_J], F32)
            q_col = cpool.tile([128, NT_J], F32)

            # ---------------- query-side prep ----------------
            negr_col = cpool.tile([128, NT_I], F32)
            nc.scalar.activation(negr_col[:], s_half, AF.Exp, scale=-(1.0 - ALPHA))
            nc.vector.tensor_scalar_mul(negr_col[:], negr_col[:], -1.0)

            b_f = wpool.tile([128, NT_I], F32, tag="bf", bufs=1)
            nc.vector.tensor_scalar(
                b_f[:], s_half, -INVG, -LO * INVG, op0=OP.mult, op1=OP.add
            )
            b_rnd = wpool.tile([128, NT_I], F32, tag="brnd", bufs=1)
            nc.vector.tensor_scalar(
                b_rnd[:], b_f[:], 8388608.0, 8388608.0,
                op0=OP.add, op1=OP.subtract,
            )
            b_cl = wpool.tile([128, NT_I], BF16, tag="bcl", bufs=1)
            nc.vector.tensor_scalar(
                b_cl[:], b_rnd[:], 0.0, float(BGRID - 1), op0=OP.max, op1=OP.min
            )
            negr16 = wpool.tile([128, NT_I], FP16, tag="negr16", bufs=1)
            nc.vector.tensor_copy(negr16[:], negr_col[:])
            r_dram = dpool.tile([HALF], FP16, name="rscr")
            nc.sync.dma_start(
                r_dram[:].rearrange("(c q) -> q c", q=128), negr16[:]
            )
            negr_row = cpool.tile([1, HALF], FP16)
            nc.sync.dma_start(
                negr_row[:], r_dram[:].rearrange("(a n) -> a n", a=1)
            )
            ones_fp = cpool.tile([1, 128], FP16)
            nc.vector.tensor_copy(ones_fp[:], ones[0:1, 0:128])

            # roundtrip through DRAM to get b as a single row [1, 4096]
            b_dram = dpool.tile([HALF], BF16, name="bscr")
            nc.sync.dma_start(
                b_dram[:].rearrange("(c q) -> q c", q=128), b_cl[:]
            )
            b_row = cpool.tile([1, HALF], BF16)
            ones_bf = cpool.tile([1, 128], BF16)
            nc.vector.tensor_copy(ones_bf[:], ones[0:1, 0:128])
            nc.sync.dma_start(b_row[:], b_dram[:].rearrange("(a n) -> a n", a=1))

            # ------- fused: exps + ST build + mask + grid matmul -------
            bankV = ppool.tile([F + 1, BGRID], F32, tag="bankV", bufs=1)
            bankQ = ppool.tile([F + 1, BGRID], F32, tag="bankQ", bufs=1)
            GRP = 16
            for jt in range(NT_J):
                if jt % GRP == 0:
                    gs = slice(jt, jt + GRP)
                    nc.scalar.activation(v_col[:, gs], d_all[:, gs], AF.Exp)
                    nc.scalar.activation(
                        q_col[:, gs], d_all[:, gs], AF.Exp, scale=ALPHA
                    )
                st_t = wpool.tile([128, AUGW], FP16, tag="stt", bufs=4)
                if jt % 4 == 3:
                    nc.scalar.activation(
                        st_t[:, 0 : F + 1], hprB3[:, jt, :], AF.Identity,
                        scale=v_col[:, jt : jt + 1],
                    )
                    nc.scalar.activation(
                        st_t[:, F + 1 : AUGW], hprB3[:, jt, :],
                        AF.Identity, scale=q_col[:, jt : jt + 1],
                    )
                else:
                    nc.vector.tensor_scalar_mul(
                        st_t[:, 0 : F + 1], hprB3[:, jt, :],
                        v_col[:, jt : jt + 1],
                    )
                    nc.vector.tensor_scalar_mul(
                        st_t[:, F + 1 : AUGW], hprB3[:, jt, :],
                        q_col[:, jt : jt + 1],
                    )
                mw = wpool.tile([128, BGRID], FP16, tag="mask", bufs=4)
                nc.vector.tensor_scalar(
                    mw[:], trow_b[:], ds3[:, jt, 0:1], None, op0=OP.is_le
                )
                st, sp = (jt == 0), (jt == NT_J - 1)
                nc.tensor.matmul(
                    bankV[:], st_t[:, 0 : F + 1], mw[:], start=st, stop=sp
                )
                nc.tensor.matmul(
                    bankQ[:], st_t[:, F + 1 : AUGW], mw[:], start=st, stop=sp
                )

            # one-hot of query buckets: oh[b, i] = (b_i == b), fp16
            # and ohr = oh * (-r_i) (pre-scaled for the D-side matmul)
            oh = cpool.tile([128, HALF], FP16)
            ohr = cpool.tile([128, HALF], FP16)
            for ch in range(8):
                br_ps = ppool.tile([128, 512], F32, tag="mix")
                nc.tensor.matmul(
                    br_ps[:],
                    ones_bf[:],
                    b_row[:, ch * 512 : (ch + 1) * 512],
                )
                nc.vector.tensor_scalar(
                    oh[:, ch * 512 : (ch + 1) * 512], br_ps[:],
                    iota_f[:, 0:1], None, op0=OP.is_equal,
                )
                nr_ps = ppool.tile([128, 512], F32, tag="mix")
                nc.tensor.matmul(
                    nr_ps[:],
                    ones_fp[:],
                    negr_row[:, ch * 512 : (ch + 1) * 512],
                )
                nc.vector.tensor_tensor(
                    ohr[:, ch * 512 : (ch + 1) * 512],
                    oh[:, ch * 512 : (ch + 1) * 512], nr_ps[:],
                    op=OP.mult,
                )

            # tables: A = g_v, D = g_q - Sq  (Sq = g_q[:, 0]); transpose to
            # [grid-part, comp] fp16 for the one-hot matmuls
            sq_col = cpool.tile([F + 1, 1], F32)
            nc.vector.tensor_copy(sq_col[:], bankQ[:, 0:1])
            A_sb = cpool.tile([F + 1, BGRID], F32)
            nc.scalar.copy(A_sb[:], bankV[:])
            D_sb = cpool.tile([F + 1, BGRID], F32)
            nc.vector.tensor_scalar(
                D_sb[:], bankQ[:], sq_col[:, 0:1], None, op0=OP.subtract
            )
            gvT_ps = ppool.tile([BGRID, F + 1], F32, tag="mix")
            nc.tensor.transpose(
                gvT_ps[:], A_sb[:],
                identity[0 : F + 1, 0 : F + 1],
            )
            gvd = cpool.tile([BGRID, AUGW], FP16)
            nc.scalar.copy(gvd[:, 0 : F + 1], gvT_ps[:])
            gdT_ps = ppool.tile([BGRID, F + 1], F32, tag="mix")
            nc.tensor.transpose(
                gdT_ps[:], D_sb[:],
                identity[0 : F + 1, 0 : F + 1],
            )
            nc.vector.tensor_copy(gdT_ps_sb_dummy := gvd[:, F + 1 : AUGW], gdT_ps[:])

            # ---------------- per-tile one-hot gather + epilogue -----------
            o_all = cpool.tile([128, NT_I * F], F32)
            o3 = o_all[:].rearrange("p (t c) -> p t c", c=F)
            out_view = out_d.ap().rearrange("(a p) f -> p a f", p=128)
            for it in range(NT_I):
                ad_ps = ppool.tile([128, F + 1], F32, tag="adps", bufs=3)
                nc.tensor.matmul(
                    ad_ps[:], oh[:, it * 128 : (it + 1) * 128],
                    gvd[:, 0 : F + 1], start=True, stop=False,
                )
                nc.tensor.matmul(
                    ad_ps[:], ohr[:, it * 128 : (it + 1) * 128],
                    gvd[:, F + 1 : AUGW], start=False, stop=True,
                )
                rec = wpool.tile([128, 1], F32, tag="rec", bufs=3)
                nc.vector.reciprocal(rec[:], ad_ps[:, F : F + 1])
                nc.vector.scalar_tensor_tensor(
                    o3[:, it, :], ad_ps[:, 0:F], rec[:, 0:1], bias_rep[:],
                    op0=OP.mult, op1=OP.add,
                )
                if it % 4 == 3:
                    grp = it // 4
                    nc.sync.dma_start(
                        out_view[:, grp * 4 : (grp + 1) * 4, :],
                        o_all[:, grp * 4 * F : (grp + 1) * 4 * F],
                    )

    nc.compile()
    return nc


_NC_CACHE = None


def _get_nc():
    global _NC_CACHE
    if _NC_CACHE is None:
        _NC_CACHE = _build_kernel_module()
    return _NC_CACHE


def _make_in_maps(h, w, a_src, a_dst, bias):
    h = np.ascontiguousarray(np.asarray(h, dtype=np.float32))
    w = np.asarray(w, dtype=np.float32)
    a_src = np.asarray(a_src, dtype=np.float32)
    a_dst = np.asarray(a_dst, dtype=np.float32)
    bias = np.asarray(bias, dtype=np.float32).reshape(1, F)
    in_maps = []
    for c in range(8):
        head, half = c // 2, c % 2
        aa = np.ascontiguousarray(
            np.concatenate([a_src[head], a_dst[head]], axis=1)
        )
        # rotate rows so this core's query half is rows 0..HALF-1
        hrot = np.ascontiguousarray(
            np.concatenate([h[half * HALF :], h[: half * HALF]])
        )
        in_maps.append(
            {
                "hfull": hrot,
                "w": np.ascontiguousarray(w[head]),
                "aa": aa,
                "bias": bias,
            }
        )
    return in_maps


def _run(h, w, a_src, a_dst, bias, trace=False, **trace_kwargs):
    nc = _get_nc()
    in_maps = _make_in_maps(h, w, a_src, a_dst, bias)
    res = run_bass_kernel_spmd(
        nc, in_maps, core_ids=list(range(8)), trace=trace, **trace_kwargs
    )
    out = np.zeros((BS, NH * F), dtype=np.float32)
    for c in range(8):
        head, half = c // 2, c % 2
        out[half * HALF : (half + 1) * HALF, head * F : (head + 1) * F] = res.results[
            c
        ]["out"]
    return out, res


def kernel(h, w, a_src, a_dst, bias):
    out, _ = _run(h, w, a_src, a_dst, bias, trace=False)
    return out


# revision 38
# speedup vs baseline: 1.2690x; 1.0753x over previous
"""Multi-head graph-attention (GAT) kernel for Trainium2, 8 NeuronCores.

Reference computation (per head):
    h_prime = h @ w[head]                       # [8192, 64]
    s = h_prime @ a_src[head],  d = h_prime @ a_dst[head]
    attn = softmax_j(leaky_relu(s_i + d_j, 0.2))
    out  = attn @ h_prime + bias                # -> [8192, 4*64]

Key identity: with exp monotone,
    exp(lrelu(s_i + d_j)) = e^{s_i} e^{d_j}           if s_i + d_j >= 0
                          = e^{0.2 s_i} e^{0.2 d_j}   otherwise
The mask sets {j : d_j >= -s_i} are NESTED across i, so the whole O(n^2)
attention contraction is a 1-D step function of t = -s_i:
    g(t) = sum_{j: d_j >= t} [v_j h'_j | v_j],   v = e^d   (and gq with q)
Evaluate g on a fixed grid of B=128 thresholds t_b (one fp16 matmul with
only B moving columns per 128-row j-tile), then each query i picks its
column b_i = clamp((-s_i - LO)/G) via a one-hot matmul that lands the
result directly in query-partition layout.  Grid rounding only
misclassifies js with |s_i + d_j| <= G, where exp(lrelu(x)) is
continuous, so the error is O(G^2) ~ 3e-4.  Everything else is exact:
    num_i / e^{s_i} = A(b_i) - r_i (C(b_i) - Sq),   r_i = e^{-0.8 s_i}
    den_i / e^{s_i} = a(b_i) - r_i (c(b_i) - sq)
with A|a = g_v columns, C|c = g_q columns, Sq|sq the full q-sums
(= g_q at the leftmost grid point).

Sharding: 8 cores = 4 heads x 2 query-halves. Each core gets the full h
(rows rotated so its query half is rows 0..4095 - j-side sums are order
invariant), computes the full j-side tables, and evaluates its 4096
queries. No collectives.
"""

import numpy as np

import concourse.bass as bass
import concourse.tile as tile
from concourse import bacc, mybir
from concourse.bass_utils import run_bass_kernel_spmd
from concourse.masks import make_identity

F32 = mybir.dt.float32
F32R = mybir.dt.float32r
BF16 = mybir.dt.bfloat16
FP16 = mybir.dt.float16
I32 = mybir.dt.int32
AF = mybir.ActivationFunctionType
OP = mybir.AluOpType

BS = 8192          # nodes
F = 64             # f_in == f_out
NH = 4             # heads
HALF = BS // 2     # queries per core
NT_J = BS // 128   # 64 j tiles
NT_I = HALF // 128 # 32 query tiles
ALPHA = 0.2

BGRID = 128        # grid points (one PE tile)
LO = -8.0          # grid range [LO, -LO)
G = (-2.0 * LO) / BGRID   # 1/8
INVG = 1.0 / G

AUGW = 2 * (F + 1)  # 130: [Hv|v | Hq|q] stationary width


def _build_kernel_module():
    nc = bacc.Bacc("TRN2", target_bir_lowering=False, debug=False)

    hfull_d = nc.dram_tensor("hfull", [BS, F], F32, kind="ExternalInput")
    w_d = nc.dram_tensor("w", [F, F], F32, kind="ExternalInput")
    aa_d = nc.dram_tensor("aa", [F, 2], F32, kind="ExternalInput")
    bias_d = nc.dram_tensor("bias", [1, F], F32, kind="ExternalInput")
    out_d = nc.dram_tensor("out", [HALF, F], F32, kind="ExternalOutput")

    with tile.TileContext(nc) as tc:
        with (
            tc.tile_pool(name="const", bufs=1) as cpool,
            tc.tile_pool(name="work", bufs=3) as wpool,
            tc.tile_pool(name="dscr", bufs=1, space="DRAM") as dpool,
            tc.tile_pool(name="psum", bufs=3, space="PSUM") as ppool,
        ):
            # ---------------- constants ----------------
            identity = cpool.tile([128, 128], F32)
            make_identity(nc, identity[:])
            ones = cpool.tile([128, 128], F32)
            nc.gpsimd.memset(ones[:], 1.0)
            iota_col = cpool.tile([128, 1], I32)
            nc.gpsimd.iota(iota_col[:], [[1, 1]], base=0, channel_multiplier=1)
            iota_f = cpool.tile([128, 1], F32)
            nc.vector.tensor_copy(iota_f[:], iota_col[:])

            # ---------------- h load, h^T (paired), h' (+d +s) -------------
            # hT2 chunk p holds transposed tiles 2p (partitions 0:64) and
            # 2p+1 (partitions 64:128).
            hT2 = cpool.tile([128, (NT_J // 2) * 128], BF16)
            # h' in bf16, padded with a ones column (aug col 64)
            hprB = cpool.tile([128, NT_J * (F + 1)], BF16)
            hprB3 = hprB[:].rearrange("p (t c) -> p t c", c=F + 1)
            nc.gpsimd.memset(hprB3[:, :, F], 1.0)
            ds_col = cpool.tile([128, NT_J * 2], F32)  # [d | s] per tile
            ds3 = ds_col[:].rearrange("p (t c) -> p t c", c=2)
            hf_view = hfull_d.ap().rearrange("(a p) f -> p a f", p=128)
            for blk in range(NT_J // 4):
                ldb = wpool.tile([128, 4 * F], F32, tag="hloadb", bufs=4)
                nc.sync.dma_start(
                    ldb[:], hf_view[:, blk * 4 : (blk + 1) * 4, :]
                )
                for k in range(2):
                    pr = blk * 2 + k  # pair index
                    tr = ppool.tile([128, 128], F32, tag="mix")
                    nc.tensor.transpose(
                        tr[:],
                        ldb[:, k * 128 : (k + 1) * 128],
                        identity[:],
                    )
                    if k % 2 == 0:
                        nc.scalar.copy(hT2[:, pr * 128 : (pr + 1) * 128], tr[:])
                    else:
                        nc.vector.tensor_copy(
                            hT2[:, pr * 128 : (pr + 1) * 128], tr[:]
                        )
            # ---------------- tiny weight prep ----------------
            w_sb = cpool.tile([F, F], F32)
            nc.sync.dma_start(w_sb[:], w_d.ap())
            aa_sb = cpool.tile([F, 2], F32)
            nc.sync.dma_start(aa_sb[:], aa_d.ap())
            bias_sb = cpool.tile([1, F], F32)
            nc.sync.dma_start(bias_sb[:], bias_d.ap())

            wT_ps = ppool.tile([F, F], F32, tag="mix")
            nc.tensor.transpose(wT_ps[:], w_sb[:], identity[0:F, 0:F])
            wT_sb = cpool.tile([F, F], F32)
            nc.scalar.copy(wT_sb[:], wT_ps[:])

            # ws = w @ [a_src | a_dst]  -> [64, 2]
            ws_ps = ppool.tile([F, 2], F32, tag="mix")
            nc.tensor.matmul(ws_ps[:], wT_sb[:], aa_sb[:])
            ws_sb = cpool.tile([F, 2], F32)
            nc.scalar.copy(ws_sb[:], ws_ps[:])

            # w_aug = [w | w@a_dst | w@a_src]: h @ w_aug -> [h' | d | s]
            # replicated on partitions 64:128 for odd-tile stationaries
            w_top = cpool.tile([F, F + 2], F32)
            nc.scalar.copy(w_top[:, 0:F], w_sb[:])
            nc.scalar.copy(w_top[:, F : F + 1], ws_sb[:, 1:2])
            nc.scalar.copy(w_top[:, F + 1 : F + 2], ws_sb[:, 0:1])
            dbl_id = cpool.tile([F, 128], F32)
            nc.scalar.copy(dbl_id[:, 0:F], identity[0:F, 0:F])
            nc.vector.tensor_copy(dbl_id[:, F:128], identity[0:F, 0:F])
            wa_ps = ppool.tile([128, F + 2], F32, tag="mix")
            nc.tensor.matmul(wa_ps[:], dbl_id[:], w_top[:])
            w_aug = cpool.tile([128, F + 2], BF16)
            nc.scalar.copy(w_aug[:], wa_ps[:])

            # bias broadcast to all partitions
            biasb_ps = ppool.tile([128, F], F32, tag="mix")
            nc.tensor.matmul(biasb_ps[:], ones[0:1, 0:128], bias_sb[:])
            bias_rep = cpool.tile([128, F], F32)
            nc.scalar.copy(bias_rep[:], biasb_ps[:])

            # grid thresholds replicated across partitions, bf16-exact
            tg_f = cpool.tile([1, BGRID], F32)
            nc.gpsimd.iota(
                tg_f[:], [[1, BGRID]], base=0, channel_multiplier=0,
                allow_small_or_imprecise_dtypes=True,
            )
            tgrid = cpool.tile([1, BGRID], F32)
            nc.vector.tensor_scalar(
                tgrid[:], tg_f[:], G, LO, op0=OP.mult, op1=OP.add
            )
            trow_ps = ppool.tile([128, BGRID], F32, tag="mix")
            nc.tensor.matmul(trow_ps[:], ones[0:1, 0:128], tgrid[:])
            trow_b = cpool.tile([128, BGRID], BF16)
            nc.scalar.copy(trow_b[:], trow_ps[:])

            for jt in range(NT_J):
                pr, par = jt // 2, (jt % 2) * F
                hp_ps = ppool.tile([128, F + 2], F32, tag="mix")
                nc.tensor.matmul(
                    hp_ps[:],
                    hT2[par : par + F, pr * 128 : (pr + 1) * 128],
                    w_aug[par : par + F, :],
                )
                if jt % 2 == 0:
                    nc.scalar.copy(hprB3[:, jt, 0:F], hp_ps[:, 0:F])
                    nc.vector.tensor_copy(ds3[:, jt, :], hp_ps[:, F : F + 2])
                else:
                    nc.vector.tensor_copy(hprB3[:, jt, 0:F], hp_ps[:, 0:F])
                    nc.scalar.copy(ds3[:, jt, :], hp_ps[:, F : F + 2])

            d_all = ds3[:, :, 0]            # [128, NT_J] strided
            s_half = ds3[:, 0:NT_I, 1]      # queries = rows 0..4095

            v_col = cpool.tile([128, NT_J], F32)
            q_col = cpool.tile([128, NT_J], F32)

            # ---------------- query-side prep ----------------
            negr_col = cpool.tile([128, NT_I], F32)
            nc.scalar.activation(negr_col[:], s_half, AF.Exp, scale=-(1.0 - ALPHA))
            nc.vector.tensor_scalar_mul(negr_col[:], negr_col[:], -1.0)

            b_f = wpool.tile([128, NT_I], F32, tag="bf", bufs=1)
            nc.vector.tensor_scalar(
                b_f[:], s_half, -INVG, -LO * INVG, op0=OP.mult, op1=OP.add
            )
            b_rnd = wpool.tile([128, NT_I], F32, tag="brnd", bufs=1)
            nc.vector.tensor_scalar(
                b_rnd[:], b_f[:], 8388608.0, 8388608.0,
                op0=OP.add, op1=OP.subtract,
            )
            b_cl = wpool.tile([128, NT_I], BF16, tag="bcl", bufs=1)
            nc.vector.tensor_scalar(
                b_cl[:], b_rnd[:], 0.0, float(BGRID - 1), op0=OP.max, op1=OP.min
            )
            negr16 = wpool.tile([128, NT_I], FP16, tag="negr16", bufs=1)
            nc.vector.tensor_copy(negr16[:], negr_col[:])
            r_dram = dpool.tile([HALF], FP16, name="rscr")
            nc.sync.dma_start(
                r_dram[:].rearrange("(c q) -> q c", q=128), negr16[:]
            )
            negr_row = cpool.tile([1, HALF], FP16)
            nc.sync.dma_start(
                negr_row[:], r_dram[:].rearrange("(a n) -> a n", a=1)
            )
            ones_fp = cpool.tile([1, 128], FP16)
            nc.vector.tensor_copy(ones_fp[:], ones[0:1, 0:128])

            # roundtrip through DRAM to get b as a single row [1, 4096]
            b_dram = dpool.tile([HALF], BF16, name="bscr")
            nc.sync.dma_start(
                b_dram[:].rearrange("(c q) -> q c", q=128), b_cl[:]
            )
            b_row = cpool.tile([1, HALF], BF16)
            ones_bf = cpool.tile([1, 128], BF16)
            nc.vector.tensor_copy(ones_bf[:], ones[0:1, 0:128])
            nc.sync.dma_start(b_row[:], b_dram[:].rearrange("(a n) -> a n", a=1))

            # ------- fused: exps + ST build + mask + grid matmul -------
            bankV = ppool.tile([F + 1, BGRID], F32, tag="bankV", bufs=1)
            bankQ = ppool.tile([F + 1, BGRID], F32, tag="bankQ", bufs=1)
            GRP = 16
            for jt in range(NT_J):
                if jt % GRP == 0:
                    gs = slice(jt, jt + GRP)
                    nc.scalar.activation(v_col[:, gs], d_all[:, gs], AF.Exp)
                    nc.scalar.activation(
                        q_col[:, gs], d_all[:, gs], AF.Exp, scale=ALPHA
                    )
                st_t = wpool.tile([128, AUGW], FP16, tag="stt", bufs=4)
                if jt % 4 == 3:
                    nc.scalar.activation(
                        st_t[:, 0 : F + 1], hprB3[:, jt, :], AF.Identity,
                        scale=v_col[:, jt : jt + 1],
                    )
                    nc.scalar.activation(
                        st_t[:, F + 1 : AUGW], hprB3[:, jt, :],
                        AF.Identity, scale=q_col[:, jt : jt + 1],
                    )
                else:
                    nc.vector.tensor_scalar_mul(
                        st_t[:, 0 : F + 1], hprB3[:, jt, :],
                        v_col[:, jt : jt + 1],
                    )
                    nc.vector.tensor_scalar_mul(
                        st_t[:, F + 1 : AUGW], hprB3[:, jt, :],
                        q_col[:, jt : jt + 1],
                    )
                mw = wpool.tile([128, BGRID], FP16, tag="mask", bufs=4)
                nc.vector.tensor_scalar(
                    mw[:], trow_b[:], ds3[:, jt, 0:1], None, op0=OP.is_le
                )
                st, sp = (jt == 0), (jt == NT_J - 1)
                nc.tensor.matmul(
                    bankV[:], st_t[:, 0 : F + 1], mw[:], start=st, stop=sp
                )
                nc.tensor.matmul(
                    bankQ[:], st_t[:, F + 1 : AUGW], mw[:], start=st, stop=sp
                )

            # tables: A = g_v, D = g_q - Sq  (Sq = g_q[:, 0]); transpose to
            # [grid-part, comp] fp16 for the one-hot matmuls
            sq_col = cpool.tile([F + 1, 1], F32)
            nc.vector.tensor_copy(sq_col[:], bankQ[:, 0:1])
            A_sb = cpool.tile([F + 1, BGRID], F32)
            nc.scalar.copy(A_sb[:], bankV[:])
            D_sb = cpool.tile([F + 1, BGRID], F32)
            nc.vector.tensor_scalar(
                D_sb[:], bankQ[:], sq_col[:, 0:1], None, op0=OP.subtract
            )
            gvT_ps = ppool.tile([BGRID, F + 1], F32, tag="mix")
            nc.tensor.transpose(
                gvT_ps[:], A_sb[:],
                identity[0 : F + 1, 0 : F + 1],
            )
            gvd = cpool.tile([BGRID, AUGW], FP16)
            nc.scalar.copy(gvd[:, 0 : F + 1], gvT_ps[:])
            gdT_ps = ppool.tile([BGRID, F + 1], F32, tag="mix")
            nc.tensor.transpose(
                gdT_ps[:], D_sb[:],
                identity[0 : F + 1, 0 : F + 1],
            )
            nc.vector.tensor_copy(gdT_ps_sb_dummy := gvd[:, F + 1 : AUGW], gdT_ps[:])

            # one-hot of query buckets: oh[b, i] = (b_i == b), fp16
            # and ohr = oh * (-r_i) (pre-scaled for the D-side matmul)
            oh = cpool.tile([128, HALF], FP16)
            ohr = cpool.tile([128, HALF], FP16)
            for ch in range(8):
                br_ps = ppool.tile([128, 512], F32, tag="mix")
                nc.tensor.matmul(
                    br_ps[:],
                    ones_bf[:],
                    b_row[:, ch * 512 : (ch + 1) * 512],
                )
                nc.vector.tensor_scalar(
                    oh[:, ch * 512 : (ch + 1) * 512], br_ps[:],
                    iota_f[:, 0:1], None, op0=OP.is_equal,
                )
                nr_ps = ppool.tile([128, 512], F32, tag="mix")
                nc.tensor.matmul(
                    nr_ps[:],
                    ones_fp[:],
                    negr_row[:, ch * 512 : (ch + 1) * 512],
                )
                nc.vector.tensor_tensor(
                    ohr[:, ch * 512 : (ch + 1) * 512],
                    oh[:, ch * 512 : (ch + 1) * 512], nr_ps[:],
                    op=OP.mult,
                )

            # ---------------- per-tile one-hot gather + epilogue -----------
            o_all = cpool.tile([128, NT_I * F], F32)
            o3 = o_all[:].rearrange("p (t c) -> p t c", c=F)
            out_view = out_d.ap().rearrange("(a p) f -> p a f", p=128)
            for it in range(NT_I):
                ad_ps = ppool.tile([128, F + 1], F32, tag="adps", bufs=3)
                nc.tensor.matmul(
                    ad_ps[:], oh[:, it * 128 : (it + 1) * 128],
                    gvd[:, 0 : F + 1], start=True, stop=False,
                )
                nc.tensor.matmul(
                    ad_ps[:], ohr[:, it * 128 : (it + 1) * 128],
                    gvd[:, F + 1 : AUGW], start=False, stop=True,
                )
                rec = wpool.tile([128, 1], F32, tag="rec", bufs=3)
                nc.vector.reciprocal(rec[:], ad_ps[:, F : F + 1])
                nc.vector.scalar_tensor_tensor(
                    o3[:, it, :], ad_ps[:, 0:F], rec[:, 0:1], bias_rep[:],
                    op0=OP.mult, op1=OP.add,
                )
                if it % 4 == 3:
                    grp = it // 4
                    nc.sync.dma_start(
                        out_view[:, grp * 4 : (grp + 1) * 4, :],
                        o_all[:, grp * 4 * F : (grp + 1) * 4 * F],
                    )

    nc.compile()
    return nc


_NC_CACHE = None


def _get_nc():
    global _NC_CACHE
    if _NC_CACHE is None:
        _NC_CACHE = _build_kernel_module()
    return _NC_CACHE


def _make_in_maps(h, w, a_src, a_dst, bias):
    h = np.ascontiguousarray(np.asarray(h, dtype=np.float32))
    w = np.asarray(w, dtype=np.float32)
    a_src = np.asarray(a_src, dtype=np.float32)
    a_dst = np.asarray(a_dst, dtype=np.float32)
    bias = np.asarray(bias, dtype=np.float32).reshape(1, F)
    in_maps = []
    for c in range(8):
        head, half = c // 2, c % 2
        aa = np.ascontiguousarray(
            np.concatenate([a_src[head], a_dst[head]], axis=1)
        )
        # rotate rows so this core's query half is rows 0..HALF-1
        hrot = np.ascontiguousarray(
            np.concatenate([h[half * HALF :], h[: half * HALF]])
        )
        in_maps.append(
            {
                "hfull": hrot,
                "w": np.ascontiguousarray(w[head]),
                "aa": aa,
                "bias": bias,
            }
        )
    return in_maps


def _run(h, w, a_src, a_dst, bias, trace=False, **trace_kwargs):
    nc = _get_nc()
    in_maps = _make_in_maps(h, w, a_src, a_dst, bias)
    res = run_bass_kernel_spmd(
        nc, in_maps, core_ids=list(range(8)), trace=trace, **trace_kwargs
    )
    out = np.zeros((BS, NH * F), dtype=np.float32)
    for c in range(8):
        head, half = c // 2, c % 2
        out[half * HALF : (half + 1) * HALF, head * F : (head + 1) * F] = res.results[
            c
        ]["out"]
    return out, res


def kernel(h, w, a_src, a_dst, bias):
    out, _ = _run(h, w, a_src, a_dst, bias, trace=False)
    return out


# revision 39
# speedup vs baseline: 1.3450x; 1.0599x over previous
"""Multi-head graph-attention (GAT) kernel for Trainium2, 8 NeuronCores.

Reference computation (per head):
    h_prime = h @ w[head]                       # [8192, 64]
    s = h_prime @ a_src[head],  d = h_prime @ a_dst[head]
    attn = softmax_j(leaky_relu(s_i + d_j, 0.2))
    out  = attn @ h_prime + bias                # -> [8192, 4*64]

Key identity: with exp monotone,
    exp(lrelu(s_i + d_j)) = e^{s_i} e^{d_j}           if s_i + d_j >= 0
                          = e^{0.2 s_i} e^{0.2 d_j}   otherwise
The mask sets {j : d_j >= -s_i} are NESTED across i, so the whole O(n^2)
attention contraction is a 1-D step function of t = -s_i:
    g(t) = sum_{j: d_j >= t} [v_j h'_j | v_j],   v = e^d   (and gq with q)
Evaluate g on a fixed grid of B=128 thresholds t_b (one fp16 matmul with
only B moving columns per 128-row j-tile), then each query i picks its
column b_i = clamp((-s_i - LO)/G) via a one-hot matmul that lands the
result directly in query-partition layout.  Grid rounding only
misclassifies js with |s_i + d_j| <= G, where exp(lrelu(x)) is
continuous, so the error is O(G^2) ~ 3e-4.  Everything else is exact:
    num_i / e^{s_i} = A(b_i) - r_i (C(b_i) - Sq),   r_i = e^{-0.8 s_i}
    den_i / e^{s_i} = a(b_i) - r_i (c(b_i) - sq)
with A|a = g_v columns, C|c = g_q columns, Sq|sq the full q-sums
(= g_q at the leftmost grid point).

Sharding: 8 cores = 4 heads x 2 query-halves. Each core gets the full h
(rows rotated so its query half is rows 0..4095 - j-side sums are order
invariant), computes the full j-side tables, and evaluates its 4096
queries. No collectives.
"""

import numpy as np

import concourse.bass as bass
import concourse.tile as tile
from concourse import bacc, mybir
from concourse.bass_utils import run_bass_kernel_spmd
from concourse.masks import make_identity

F32 = mybir.dt.float32
F32R = mybir.dt.float32r
BF16 = mybir.dt.bfloat16
FP16 = mybir.dt.float16
I32 = mybir.dt.int32
AF = mybir.ActivationFunctionType
OP = mybir.AluOpType

BS = 8192          # nodes
F = 64             # f_in == f_out
NH = 4             # heads
HALF = BS // 2     # queries per core
NT_J = BS // 128   # 64 j tiles
NT_I = HALF // 128 # 32 query tiles
ALPHA = 0.2

BGRID = 128        # grid points (one PE tile)
LO = -8.0          # grid range [LO, -LO)
G = (-2.0 * LO) / BGRID   # 1/8
INVG = 1.0 / G

AUGW = 2 * (F + 1)  # 130: [Hv|v | Hq|q] stationary width


def _build_kernel_module():
    nc = bacc.Bacc("TRN2", target_bir_lowering=False, debug=False)

    hfull_d = nc.dram_tensor("hfull", [BS, F], F32, kind="ExternalInput")
    w_d = nc.dram_tensor("w", [F, F], F32, kind="ExternalInput")
    aa_d = nc.dram_tensor("aa", [F, 2], F32, kind="ExternalInput")
    bias_d = nc.dram_tensor("bias", [1, F], F32, kind="ExternalInput")
    out_d = nc.dram_tensor("out", [HALF, F], F32, kind="ExternalOutput")

    with tile.TileContext(nc) as tc:
        with (
            tc.tile_pool(name="const", bufs=1) as cpool,
            tc.tile_pool(name="work", bufs=3) as wpool,
            tc.tile_pool(name="dscr", bufs=1, space="DRAM") as dpool,
            tc.tile_pool(name="psum", bufs=3, space="PSUM") as ppool,
        ):
            # ---------------- constants ----------------
            identity = cpool.tile([128, 128], F32)
            make_identity(nc, identity[:])
            ones = cpool.tile([128, 128], F32)
            nc.gpsimd.memset(ones[:], 1.0)
            iota_col = cpool.tile([128, 1], I32)
            nc.gpsimd.iota(iota_col[:], [[1, 1]], base=0, channel_multiplier=1)
            iota_f = cpool.tile([128, 1], F32)
            nc.vector.tensor_copy(iota_f[:], iota_col[:])

            # ---------------- h load, h^T (paired), h' (+d +s) -------------
            # hT2 chunk p holds transposed tiles 2p (partitions 0:64) and
            # 2p+1 (partitions 64:128).
            hT2 = cpool.tile([128, (NT_J // 2) * 128], BF16)
            # h' in bf16, padded with a ones column (aug col 64)
            hprB = cpool.tile([128, NT_J * (F + 1)], BF16)
            hprB3 = hprB[:].rearrange("p (t c) -> p t c", c=F + 1)
            nc.gpsimd.memset(hprB3[:, :, F], 1.0)
            ds_col = cpool.tile([128, NT_J * 2], F32)  # [d | s] per tile
            ds3 = ds_col[:].rearrange("p (t c) -> p t c", c=2)
            hf_view = hfull_d.ap().rearrange("(a p) f -> p a f", p=128)
            for blk in range(NT_J // 8):
                ldb = wpool.tile([128, 8 * F], F32, tag="hloadb", bufs=3)
                nc.sync.dma_start(
                    ldb[:], hf_view[:, blk * 8 : (blk + 1) * 8, :]
                )
                for k in range(4):
                    pr = blk * 4 + k  # pair index
                    tr = ppool.tile([128, 128], F32, tag="mix")
                    nc.tensor.transpose(
                        tr[:],
                        ldb[:, k * 128 : (k + 1) * 128],
                        identity[:],
                    )
                    if k % 2 == 0:
                        nc.scalar.copy(hT2[:, pr * 128 : (pr + 1) * 128], tr[:])
                    else:
                        nc.vector.tensor_copy(
                            hT2[:, pr * 128 : (pr + 1) * 128], tr[:]
                        )
            # ---------------- tiny weight prep ----------------
            w_sb = cpool.tile([F, F], F32)
            nc.sync.dma_start(w_sb[:], w_d.ap())
            aa_sb = cpool.tile([F, 2], F32)
            nc.sync.dma_start(aa_sb[:], aa_d.ap())
            bias_sb = cpool.tile([1, F], F32)
            nc.sync.dma_start(bias_sb[:], bias_d.ap())

            wT_ps = ppool.tile([F, F], F32, tag="mix")
            nc.tensor.transpose(wT_ps[:], w_sb[:], identity[0:F, 0:F])
            wT_sb = cpool.tile([F, F], F32)
            nc.scalar.copy(wT_sb[:], wT_ps[:])

            # ws = w @ [a_src | a_dst]  -> [64, 2]
            ws_ps = ppool.tile([F, 2], F32, tag="mix")
            nc.tensor.matmul(ws_ps[:], wT_sb[:], aa_sb[:])
            ws_sb = cpool.tile([F, 2], F32)
            nc.scalar.copy(ws_sb[:], ws_ps[:])

            # w_aug = [w | w@a_dst | w@a_src]: h @ w_aug -> [h' | d | s]
            # replicated on partitions 64:128 for odd-tile stationaries
            w_top = cpool.tile([F, F + 2], F32)
            nc.scalar.copy(w_top[:, 0:F], w_sb[:])
            nc.scalar.copy(w_top[:, F : F + 1], ws_sb[:, 1:2])
            nc.scalar.copy(w_top[:, F + 1 : F + 2], ws_sb[:, 0:1])
            dbl_id = cpool.tile([F, 128], F32)
            nc.scalar.copy(dbl_id[:, 0:F], identity[0:F, 0:F])
            nc.vector.tensor_copy(dbl_id[:, F:128], identity[0:F, 0:F])
            wa_ps = ppool.tile([128, F + 2], F32, tag="mix")
            nc.tensor.matmul(wa_ps[:], dbl_id[:], w_top[:])
            w_aug = cpool.tile([128, F + 2], BF16)
            nc.scalar.copy(w_aug[:], wa_ps[:])

            # bias broadcast to all partitions
            biasb_ps = ppool.tile([128, F], F32, tag="mix")
            nc.tensor.matmul(biasb_ps[:], ones[0:1, 0:128], bias_sb[:])
            bias_rep = cpool.tile([128, F], F32)
            nc.scalar.copy(bias_rep[:], biasb_ps[:])

            # grid thresholds replicated across partitions, bf16-exact
            tg_f = cpool.tile([1, BGRID], F32)
            nc.gpsimd.iota(
                tg_f[:], [[1, BGRID]], base=0, channel_multiplier=0,
                allow_small_or_imprecise_dtypes=True,
            )
            tgrid = cpool.tile([1, BGRID], F32)
            nc.vector.tensor_scalar(
                tgrid[:], tg_f[:], G, LO, op0=OP.mult, op1=OP.add
            )
            trow_ps = ppool.tile([128, BGRID], F32, tag="mix")
            nc.tensor.matmul(trow_ps[:], ones[0:1, 0:128], tgrid[:])
            trow_b = cpool.tile([128, BGRID], BF16)
            nc.scalar.copy(trow_b[:], trow_ps[:])

            for jt in range(NT_J):
                pr, par = jt // 2, (jt % 2) * F
                hp_ps = ppool.tile([128, F + 2], F32, tag="mix")
                nc.tensor.matmul(
                    hp_ps[:],
                    hT2[par : par + F, pr * 128 : (pr + 1) * 128],
                    w_aug[par : par + F, :],
                )
                if jt % 2 == 0:
                    nc.scalar.copy(hprB3[:, jt, 0:F], hp_ps[:, 0:F])
                    nc.vector.tensor_copy(ds3[:, jt, :], hp_ps[:, F : F + 2])
                else:
                    nc.vector.tensor_copy(hprB3[:, jt, 0:F], hp_ps[:, 0:F])
                    nc.scalar.copy(ds3[:, jt, :], hp_ps[:, F : F + 2])

            d_all = ds3[:, :, 0]            # [128, NT_J] strided
            s_half = ds3[:, 0:NT_I, 1]      # queries = rows 0..4095

            v_col = cpool.tile([128, NT_J], F32)
            q_col = cpool.tile([128, NT_J], F32)

            # ---------------- query-side prep ----------------
            negr_col = cpool.tile([128, NT_I], F32)
            nc.scalar.activation(negr_col[:], s_half, AF.Exp, scale=-(1.0 - ALPHA))
            nc.vector.tensor_scalar_mul(negr_col[:], negr_col[:], -1.0)

            b_f = wpool.tile([128, NT_I], F32, tag="bf", bufs=1)
            nc.vector.tensor_scalar(
                b_f[:], s_half, -INVG, -LO * INVG, op0=OP.mult, op1=OP.add
            )
            b_rnd = wpool.tile([128, NT_I], F32, tag="brnd", bufs=1)
            nc.vector.tensor_scalar(
                b_rnd[:], b_f[:], 8388608.0, 8388608.0,
                op0=OP.add, op1=OP.subtract,
            )
            b_cl = wpool.tile([128, NT_I], BF16, tag="bcl", bufs=1)
            nc.vector.tensor_scalar(
                b_cl[:], b_rnd[:], 0.0, float(BGRID - 1), op0=OP.max, op1=OP.min
            )
            negr16 = wpool.tile([128, NT_I], FP16, tag="negr16", bufs=1)
            nc.vector.tensor_copy(negr16[:], negr_col[:])
            r_dram = dpool.tile([HALF], FP16, name="rscr")
            nc.sync.dma_start(
                r_dram[:].rearrange("(c q) -> q c", q=128), negr16[:]
            )
            negr_row = cpool.tile([1, HALF], FP16)
            nc.sync.dma_start(
                negr_row[:], r_dram[:].rearrange("(a n) -> a n", a=1)
            )
            ones_fp = cpool.tile([1, 128], FP16)
            nc.vector.tensor_copy(ones_fp[:], ones[0:1, 0:128])

            # roundtrip through DRAM to get b as a single row [1, 4096]
            b_dram = dpool.tile([HALF], BF16, name="bscr")
            nc.sync.dma_start(
                b_dram[:].rearrange("(c q) -> q c", q=128), b_cl[:]
            )
            b_row = cpool.tile([1, HALF], BF16)
            ones_bf = cpool.tile([1, 128], BF16)
            nc.vector.tensor_copy(ones_bf[:], ones[0:1, 0:128])
            nc.sync.dma_start(b_row[:], b_dram[:].rearrange("(a n) -> a n", a=1))

            # ------- fused: exps + ST build + mask + grid matmul -------
            bankV = ppool.tile([F + 1, BGRID], F32, tag="bankV", bufs=1)
            bankQ = ppool.tile([F + 1, BGRID], F32, tag="bankQ", bufs=1)
            GRP = 16
            for jt in range(NT_J):
                if jt % GRP == 0:
                    gs = slice(jt, jt + GRP)
                    nc.scalar.activation(v_col[:, gs], d_all[:, gs], AF.Exp)
                    nc.scalar.activation(
                        q_col[:, gs], d_all[:, gs], AF.Exp, scale=ALPHA
                    )
                st_t = wpool.tile([128, AUGW], FP16, tag="stt", bufs=4)
                if jt % 4 == 3:
                    nc.scalar.activation(
                        st_t[:, 0 : F + 1], hprB3[:, jt, :], AF.Identity,
                        scale=v_col[:, jt : jt + 1],
                    )
                    nc.scalar.activation(
                        st_t[:, F + 1 : AUGW], hprB3[:, jt, :],
                        AF.Identity, scale=q_col[:, jt : jt + 1],
                    )
                else:
                    nc.vector.tensor_scalar_mul(
                        st_t[:, 0 : F + 1], hprB3[:, jt, :],
                        v_col[:, jt : jt + 1],
                    )
                    nc.vector.tensor_scalar_mul(
                        st_t[:, F + 1 : AUGW], hprB3[:, jt, :],
                        q_col[:, jt : jt + 1],
                    )
                mw = wpool.tile([128, BGRID], FP16, tag="mask", bufs=4)
                nc.vector.tensor_scalar(
                    mw[:], trow_b[:], ds3[:, jt, 0:1], None, op0=OP.is_le
                )
                st, sp = (jt == 0), (jt == NT_J - 1)
                nc.tensor.matmul(
                    bankV[:], st_t[:, 0 : F + 1], mw[:], start=st, stop=sp
                )
                nc.tensor.matmul(
                    bankQ[:], st_t[:, F + 1 : AUGW], mw[:], start=st, stop=sp
                )

            # tables: A = g_v, D = g_q - Sq  (Sq = g_q[:, 0]); transpose to
            # [grid-part, comp] fp16 for the one-hot matmuls
            sq_col = cpool.tile([F + 1, 1], F32)
            nc.vector.tensor_copy(sq_col[:], bankQ[:, 0:1])
            A_sb = cpool.tile([F + 1, BGRID], F32)
            nc.scalar.copy(A_sb[:], bankV[:])
            D_sb = cpool.tile([F + 1, BGRID], F32)
            nc.vector.tensor_scalar(
                D_sb[:], bankQ[:], sq_col[:, 0:1], None, op0=OP.subtract
            )
            gvT_ps = ppool.tile([BGRID, F + 1], F32, tag="mix")
            nc.tensor.transpose(
                gvT_ps[:], A_sb[:],
                identity[0 : F + 1, 0 : F + 1],
            )
            gvd = cpool.tile([BGRID, AUGW], FP16)
            nc.scalar.copy(gvd[:, 0 : F + 1], gvT_ps[:])
            gdT_ps = ppool.tile([BGRID, F + 1], F32, tag="mix")
            nc.tensor.transpose(
                gdT_ps[:], D_sb[:],
                identity[0 : F + 1, 0 : F + 1],
            )
            nc.vector.tensor_copy(gdT_ps_sb_dummy := gvd[:, F + 1 : AUGW], gdT_ps[:])

            # one-hot of query buckets: oh[b, i] = (b_i == b), fp16
            # and ohr = oh * (-r_i) (pre-scaled for the D-side matmul)
            oh = cpool.tile([128, HALF], FP16)
            ohr = cpool.tile([128, HALF], FP16)
            for ch in range(8):
                br_ps = ppool.tile([128, 512], F32, tag="mix")
                nc.tensor.matmul(
                    br_ps[:],
                    ones_bf[:],
                    b_row[:, ch * 512 : (ch + 1) * 512],
                )
                nc.vector.tensor_scalar(
                    oh[:, ch * 512 : (ch + 1) * 512], br_ps[:],
                    iota_f[:, 0:1], None, op0=OP.is_equal,
                )
                nr_ps = ppool.tile([128, 512], F32, tag="mix")
                nc.tensor.matmul(
                    nr_ps[:],
                    ones_fp[:],
                    negr_row[:, ch * 512 : (ch + 1) * 512],
                )
                nc.vector.tensor_tensor(
                    ohr[:, ch * 512 : (ch + 1) * 512],
                    oh[:, ch * 512 : (ch + 1) * 512], nr_ps[:],
                    op=OP.mult,
                )

            # ---------------- per-tile one-hot gather + epilogue -----------
            o_all = cpool.tile([128, NT_I * F], F32)
            o3 = o_all[:].rearrange("p (t c) -> p t c", c=F)
            out_view = out_d.ap().rearrange("(a p) f -> p a f", p=128)
            for it in range(NT_I):
                ad_ps = ppool.tile([128, F + 1], F32, tag="adps", bufs=3)
                nc.tensor.matmul(
                    ad_ps[:], oh[:, it * 128 : (it + 1) * 128],
                    gvd[:, 0 : F + 1], start=True, stop=False,
                )
                nc.tensor.matmul(
                    ad_ps[:], ohr[:, it * 128 : (it + 1) * 128],
                    gvd[:, F + 1 : AUGW], start=False, stop=True,
                )
                rec = wpool.tile([128, 1], F32, tag="rec", bufs=3)
                nc.vector.reciprocal(rec[:], ad_ps[:, F : F + 1])
                nc.vector.scalar_tensor_tensor(
                    o3[:, it, :], ad_ps[:, 0:F], rec[:, 0:1], bias_rep[:],
                    op0=OP.mult, op1=OP.add,
                )
                if it % 4 == 3:
                    grp = it // 4
                    nc.sync.dma_start(
                        out_view[:, grp * 4 : (grp + 1) * 4, :],
                        o_all[:, grp * 4 * F : (grp + 1) * 4 * F],
                    )

    nc.compile()
    return nc


_NC_CACHE = None


def _get_nc():
    global _NC_CACHE
    if _NC_CACHE is None:
        _NC_CACHE = _build_kernel_module()
    return _NC_CACHE


def _make_in_maps(h, w, a_src, a_dst, bias):
    h = np.ascontiguousarray(np.asarray(h, dtype=np.float32))
    w = np.asarray(w, dtype=np.float32)
    a_src = np.asarray(a_src, dtype=np.float32)
    a_dst = np.asarray(a_dst, dtype=np.float32)
    bias = np.asarray(bias, dtype=np.float32).reshape(1, F)
    in_maps = []
    for c in range(8):
        head, half = c // 2, c % 2
        aa = np.ascontiguousarray(
            np.concatenate([a_src[head], a_dst[head]], axis=1)
        )
        # rotate rows so this core's query half is rows 0..HALF-1
        hrot = np.ascontiguousarray(
            np.concatenate([h[half * HALF :], h[: half * HALF]])
        )
        in_maps.append(
            {
                "hfull": hrot,
                "w": np.ascontiguousarray(w[head]),
                "aa": aa,
                "bias": bias,
            }
        )
    return in_maps


def _run(h, w, a_src, a_dst, bias, trace=False, **trace_kwargs):
    nc = _get_nc()
    in_maps = _make_in_maps(h, w, a_src, a_dst, bias)
    res = run_bass_kernel_spmd(
        nc, in_maps, core_ids=list(range(8)), trace=trace, **trace_kwargs
    )
    out = np.zeros((BS, NH * F), dtype=np.float32)
    for c in range(8):
        head, half = c // 2, c % 2
        out[half * HALF : (half + 1) * HALF, head * F : (head + 1) * F] = res.results[
            c
        ]["out"]
    return out, res


def kernel(h, w, a_src, a_dst, bias):
    out, _ = _run(h, w, a_src, a_dst, bias, trace=False)
    return out
